# revision 9
# baseline (speedup 1.0000x reference)
"""Trainium2 Bass kernel: 4-layer dense transformer (Megatron TP over 8 NeuronCores).

Sharding (per sharding hint):
  - qkv_w / up_w sharded along output dim, out_w / down_w along input dim.
  - Heads: 16 heads / 8 cores = 2 heads per core.
  - vocab_w sharded along vocab dim (4000 cols/core); host concatenates logits.
  - Norms + residual replicated; deltas AllReduced (bf16) after out-proj / down-proj.

Device schedule: fully software-pipelined. The sequence dim is split into GN=4
groups (the causal attention q-chunks). Each phase (attn / ffn / final norm) is
a stream of per-group macro-steps; the serial prologue chain for group g+1
(AllReduce collect -> residual add -> sum-of-squares -> rsqrt -> diag(scale))
is issued one macro-step EARLY, so it runs on the vector/scalar/gpsimd engines
while the tensor engine is busy with group g's matmuls. Attention consumption
lags production by one group (attention(g-1) is issued between qkv(g) and the
q/k transposes of g) so the rope vector work never stalls the PE. This keeps
the PE continuously fed, which also keeps it at max p-state (2.4 GHz vs 1.2).
"""

import os
import sys

if "/opt/trn_rl_repo" not in sys.path:
    sys.path.insert(0, "/opt/trn_rl_repo")

import math
from dataclasses import dataclass

import numpy as np
import ml_dtypes

import concourse.bass as bass
import concourse.bacc as bacc
import concourse.mybir as mybir
import concourse.tile as tile
from concourse.bass import IndirectOffsetOnAxis
from concourse.bass_utils import run_bass_kernel_spmd
from concourse.masks import make_identity

F32 = mybir.dt.float32
BF16 = mybir.dt.bfloat16
I32 = mybir.dt.int32
AF = mybir.ActivationFunctionType
OP = mybir.AluOpType

P = 128


@dataclass(frozen=True)
class Cfg:
    S: int = 2048        # sequence
    E: int = 1024        # embed
    D: int = 64          # head dim
    HC: int = 2          # heads per core
    HIDC: int = 512      # hidden shard per core (of 4096 / 8)
    VC: int = 4000       # vocab shard per core
    L: int = 4           # layers
    n_cores: int = 8
    V_ROWS: int = 32000  # embedding table rows
    eps: float = 1e-5

    @property
    def ST(self):
        return self.S // P

    @property
    def ET(self):
        return self.E // P

    @property
    def HT(self):
        return self.HIDC // P

    @property
    def QKVC(self):
        return 3 * self.HC * self.D

    @property
    def QC(self):
        return min(512, self.S)  # q chunk for attention

    @property
    def VCHUNK(self):
        return 250


def build_kernel(cfg: Cfg):
    c = cfg
    ST, ET, HT = c.ST, c.ET, c.HT
    D, HC = c.D, c.HC
    D2 = D // 2
    EG = min(4, ET)
    GN = max(c.S // c.QC, 1)   # 4 groups
    GS = ST // GN              # 4 s-tiles per group
    EW = min(512, c.E)

    nc = bacc.Bacc(
        "TRN2", target_bir_lowering=False, debug=False, num_devices=cfg.n_cores
    )

    idx_d = nc.declare_dram_parameter("idx", [P, ST], I32, isOutput=False)
    table_d = nc.declare_dram_parameter("table", [c.V_ROWS, c.E], BF16, isOutput=False)
    qkvw_d = nc.declare_dram_parameter("qkvw", [c.L, c.E, c.QKVC], BF16, isOutput=False)
    outw_d = nc.declare_dram_parameter("outw", [c.L, HC * D, c.E], BF16, isOutput=False)
    upw_d = nc.declare_dram_parameter("upw", [c.L, c.E, 2 * c.HIDC], BF16, isOutput=False)
    downw_d = nc.declare_dram_parameter("downw", [c.L, c.HIDC, c.E], BF16, isOutput=False)
    vocw_d = nc.declare_dram_parameter("vocw", [c.E, c.VC], BF16, isOutput=False)
    cos_d = nc.declare_dram_parameter("cos", [P, ST * D], BF16, isOutput=False)
    sin_d = nc.declare_dram_parameter("sin", [P, ST * D], BF16, isOutput=False)
    out_d = nc.declare_dram_parameter("out", [c.S, c.VC], BF16, isOutput=True)

    from contextlib import ExitStack

    with tile.TileContext(nc) as tc, ExitStack() as es:
        const = es.enter_context(tc.tile_pool(name="const", bufs=1))
        wpool = es.enter_context(tc.tile_pool(name="w", bufs=1))
        act = es.enter_context(tc.tile_pool(name="act", bufs=1))
        act2 = es.enter_context(tc.tile_pool(name="act2", bufs=1))
        work = es.enter_context(tc.tile_pool(name="work", bufs=2))
        psum = es.enter_context(tc.tile_pool(name="psum", bufs=5, space="PSUM"))
        psum_tr = es.enter_context(tc.tile_pool(name="psumtr", bufs=3, space="PSUM"))
        dram = es.enter_context(tc.tile_pool(name="dram", bufs=8, space="DRAM"))

        ident = const.tile([P, P], BF16, tag="ident")
        make_identity(nc, ident[:])
        rope_tiles = {}
        for name, d in (("cos", cos_d), ("sin", sin_d)):
            t = const.tile([P, ST, D], BF16, tag=name, name=name)
            nc.sync.dma_start(t[:], d[:].rearrange("p (st d) -> p st d", d=D))
            rope_tiles[name] = t
        cosw, sinw = rope_tiles["cos"], rope_tiles["sin"]
        idx_sb = const.tile([P, ST], I32, tag="idx")
        nc.sync.dma_start(idx_sb[:], idx_d[:])

        emb = act.tile([P, ST, c.E], BF16, tag="emb")
        for st in range(ST):
            nc.gpsimd.indirect_dma_start(
                out=emb[:, st, :],
                out_offset=None,
                in_=table_d[:],
                in_offset=IndirectOffsetOnAxis(ap=idx_sb[:, st : st + 1], axis=0),
            )

        # ---------------- helpers ----------------

        def norm_prologue(g, arout_src):
            """Collect AR delta for group g into emb, compute the rmsnorm
            scale, and build diag(scale) blocks. Issued one macro-step before
            the tensor-engine consumer (norm_transposes)."""
            gsl = slice(g * GS, (g + 1) * GS)
            if arout_src is not None:
                gath = act2.tile([P, GS, c.E], BF16, tag="dg", name="gath", bufs=2)
                nc.sync.dma_start(
                    gath[:],
                    arout_src[:].rearrange("(st p) e -> p st e", p=P),
                )
                nc.vector.tensor_tensor(
                    out=emb[:, gsl, :], in0=emb[:, gsl, :], in1=gath[:], op=OP.add
                )
            ssq_g = work.tile([P, GS], F32, tag="ssqg", name="ssq_g")
            for si in range(GS):
                st = g * GS + si
                scr = work.tile([P, c.E], BF16, tag="sqscr", name="scr", bufs=1)
                nc.gpsimd.tensor_tensor(
                    out=scr[:], in0=emb[:, st, :], in1=emb[:, st, :], op=OP.mult
                )
                nc.vector.tensor_reduce(
                    out=ssq_g[:, si : si + 1], in_=scr[:],
                    axis=mybir.AxisListType.X, op=OP.add,
                )
            ms = work.tile([P, GS], F32, tag="ms", name="ms")
            nc.vector.tensor_scalar(
                ms[:], ssq_g[:], 1.0 / c.E, c.eps, OP.mult, OP.add
            )
            rms = work.tile([P, GS], F32, tag="rms", name="rms")
            nc.scalar.activation(out=rms[:], in_=ms[:], func=AF.Sqrt)
            scale = work.tile([P, GS], F32, tag="scale", name="scale")
            nc.vector.reciprocal(out=scale[:], in_=rms[:])
            scale_b = work.tile([P, GS], BF16, tag="scalebg", name="scale_b")
            nc.vector.tensor_copy(out=scale_b[:], in_=scale[:])
            diag = work.tile([P, GS, P], BF16, tag="diag", name="diag", bufs=2)
            for si in range(GS):
                nc.gpsimd.affine_select(
                    out=diag[:, si, :],
                    in_=scale_b[:, si : si + 1].to_broadcast((P, P)),
                    pattern=[[-1, P]],
                    compare_op=OP.is_equal,
                    fill=0.0,
                    base=0,
                    channel_multiplier=1,
                )
            return diag

        def norm_transposes(g, normT, diag):
            """PE-transpose emb s-tiles of group g with the rmsnorm scale
            folded in via diag(scale)."""
            for si in range(GS):
                st = g * GS + si
                for eg in range(ET // EG):
                    ptr = psum_tr.tile([P, 512], F32, tag="tr", name="tr")[:, : EG * P]
                    for j in range(EG):
                        ee = eg * EG + j
                        nc.tensor.matmul(
                            ptr[:, j * P : (j + 1) * P],
                            lhsT=emb[:, st, ee * P : (ee + 1) * P],
                            rhs=diag[:, si, :],
                            start=True,
                            stop=True,
                        )
                    nc.vector.tensor_copy(
                        out=normT[:, eg * EG : (eg + 1) * EG,
                                  st * P : (st + 1) * P],
                        in_=ptr.rearrange("p (g q) -> p g q", g=EG),
                    )

        def qkv_rope_block(g, normT, qkvw, vaug):
            qkvn = act.tile([P, GS, c.QKVC], BF16, tag="qg", name="qkvn", bufs=2)
            rq = act.tile([P, GS, HC * D], BF16, tag="ra", name="rq", bufs=2)
            rk = act.tile([P, GS, HC * D], BF16, tag="rb", name="rk", bufs=2)
            HGS = GS // 2
            for half in range(2):
                for si in range(half * HGS, (half + 1) * HGS):
                    st = g * GS + si
                    pq = psum.tile([P, 512], F32, tag="mm", name="mm")[:, : c.QKVC]
                    for kk in range(ET):
                        nc.tensor.matmul(
                            pq,
                            lhsT=normT[:, kk, st * P : (st + 1) * P],
                            rhs=qkvw[:, kk, :],
                            start=(kk == 0),
                            stop=(kk == ET - 1),
                        )
                    nc.scalar.copy(out=qkvn[:, si, :], in_=pq)
                hl = slice(half * HGS, (half + 1) * HGS)
                hg = slice(g * GS + half * HGS, g * GS + (half + 1) * HGS)
                for (src0, dst) in ((0, rq), (HC * D, rk)):
                    for h in range(HC):
                        s0 = src0 + h * D
                        tmp = work.tile([P, HGS, D], BF16, tag="ropetmp",
                                        name="tmp", bufs=2)
                        nc.vector.tensor_tensor(
                            out=tmp[:, :, 0:D2],
                            in0=qkvn[:, hl, s0 + D2 : s0 + D],
                            in1=sinw[:, hg, 0:D2],
                            op=OP.mult,
                        )
                        nc.vector.tensor_tensor(
                            out=tmp[:, :, D2:D],
                            in0=qkvn[:, hl, s0 : s0 + D2],
                            in1=sinw[:, hg, D2:D],
                            op=OP.mult,
                        )
                        o = dst[:, hl, h * D : (h + 1) * D]
                        nc.vector.tensor_tensor(
                            out=o, in0=qkvn[:, hl, s0 : s0 + D],
                            in1=cosw[:, hg], op=OP.mult)
                        nc.vector.tensor_tensor(out=o, in0=o, in1=tmp[:], op=OP.add)
                for h in range(HC):
                    nc.vector.tensor_copy(
                        out=vaug[:, hg, h * (D + 1) : h * (D + 1) + D],
                        in_=qkvn[:, hl,
                                 2 * HC * D + h * D : 2 * HC * D + (h + 1) * D],
                    )
            return rq, rk

        def qkT_block(g, rq, rk, rqT, rkT):
            for si in range(GS):
                st = g * GS + si
                ptr = psum_tr.tile([P, 512], F32, tag="tr", name="tr")
                for h in range(HC):
                    nc.tensor.matmul(
                        ptr[0:D, h * P : (h + 1) * P],
                        lhsT=rq[:, si, h * D : (h + 1) * D],
                        rhs=ident[:], start=True, stop=True)
                    nc.tensor.matmul(
                        ptr[0:D, (HC + h) * P : (HC + h + 1) * P],
                        lhsT=rk[:, si, h * D : (h + 1) * D],
                        rhs=ident[:], start=True, stop=True)
                for h in range(HC):
                    nc.vector.tensor_copy(
                        out=rqT[0:D, h, st * P : (st + 1) * P],
                        in_=ptr[0:D, h * P : (h + 1) * P])
                    nc.vector.tensor_copy(
                        out=rkT[0:D, h, st * P : (st + 1) * P],
                        in_=ptr[0:D, (HC + h) * P : (HC + h + 1) * P])

        def ar_issue_one(delta_g):
            arin = dram.tile([GS * P, c.E], BF16, tag="arin", name="arin")
            nc.sync.dma_start(
                arin[:].rearrange("(st p) e -> p st e", p=P), delta_g[:]
            )
            arout = dram.tile(
                [GS * P, c.E], BF16, tag="arout", name="arout",
                addr_space="Shared" if c.n_cores > 4 else "Local",
            )
            nc.gpsimd.collective_compute(
                "AllReduce",
                OP.add,
                replica_groups=[list(range(c.n_cores))],
                ins=[arin[:].opt()],
                outs=[arout[:].opt()],
            )
            return arout

        def attention_block(g, rqT, rkT, vaug, atto, attoT, outw, arouts):
            """scores + softmax + AV + out-proj + AR issue for q-group g.
            Both heads' score matmuls are issued before either head's AV so
            the Exp/mask chain of head h overlaps the PE work of head 1-h."""
            q0 = g * c.QC
            nk = (q0 + c.QC) // P
            PTs = []
            for h in range(HC):
                PT = act.tile([P, ST, c.QC], BF16, tag=f"pt{h}", name=f"pt{h}")
                PTs.append(PT)
                for kt in range(nk):
                    ps = psum.tile([P, 512], F32, tag="mm", name="mm")[:, : c.QC]
                    nc.tensor.matmul(
                        ps,
                        lhsT=rkT[:, h, kt * P : (kt + 1) * P],
                        rhs=rqT[:, h, q0 : q0 + c.QC],
                        start=True,
                        stop=True,
                    )
                    nc.scalar.activation(out=PT[:, kt, :], in_=ps, func=AF.Exp)
                    if (kt + 1) * P > q0:
                        nc.gpsimd.affine_select(
                            out=PT[:, kt, :],
                            in_=PT[:, kt, :],
                            pattern=[[1, c.QC]],
                            compare_op=OP.is_ge,
                            fill=0.0,
                            base=q0 - kt * P,
                            channel_multiplier=-1,
                        )
            for h in range(HC):
                PT = PTs[h]
                for qs in range(c.QC // P):
                    po = psum.tile([P, 512], F32, tag="mm", name="mm")[:, : D + 1]
                    for kt in range(nk):
                        nc.tensor.matmul(
                            po,
                            lhsT=PT[:, kt, qs * P : (qs + 1) * P],
                            rhs=vaug[:, kt, h * (D + 1) : (h + 1) * (D + 1)],
                            start=(kt == 0),
                            stop=(kt == nk - 1),
                        )
                    st_out = (q0 // P) + qs
                    rec = work.tile([P, 1], F32, tag="rec", name="rec", bufs=1)
                    nc.vector.reciprocal(out=rec[:], in_=po[:, D : D + 1])
                    nc.vector.tensor_scalar_mul(
                        atto[:, st_out, h * D : (h + 1) * D], po[:, 0:D], rec[:]
                    )
            # attoT + out-proj + AR issue
            delta_g = act2.tile([P, GS, c.E], BF16, tag="dg", name="dg", bufs=2)
            for si in range(GS):
                st = g * GS + si
                ptr = psum_tr.tile([P, 512], F32, tag="tr", name="tr")[:, :P]
                nc.tensor.matmul(ptr, lhsT=atto[:, st, :], rhs=ident[:],
                                 start=True, stop=True)
                nc.vector.tensor_copy(
                    out=attoT[:, st * P : (st + 1) * P], in_=ptr
                )
                for he in range(c.E // EW):
                    pd = psum.tile([P, 512], F32, tag="mm", name="mm")[:, :EW]
                    nc.tensor.matmul(
                        pd,
                        lhsT=attoT[:, st * P : (st + 1) * P],
                        rhs=outw[:, he * EW : (he + 1) * EW],
                        start=True,
                        stop=True,
                    )
                    nc.scalar.copy(
                        out=delta_g[:, si, he * EW : (he + 1) * EW], in_=pd
                    )
            arouts.append(ar_issue_one(delta_g))

        def ffn_up_block(g, normT2, upw, gsil, hT):
            sc0 = g * GS * P
            scw = GS * P
            for ct in range(2 * HT):
                pu = psum.tile([P, 512], F32, tag="mm", name="mm")[:, :scw]
                for kk in range(ET):
                    nc.tensor.matmul(
                        pu,
                        lhsT=upw[:, kk, ct * P : (ct + 1) * P],
                        rhs=normT2[:, kk, sc0 : sc0 + scw],
                        start=(kk == 0),
                        stop=(kk == ET - 1),
                    )
                if ct < HT:
                    sg = work.tile([P, scw], BF16, tag="sg", name="sg", bufs=1)
                    nc.scalar.activation(out=sg[:], in_=pu, func=AF.Sigmoid)
                    nc.vector.tensor_tensor(
                        out=gsil[:, ct, sc0 : sc0 + scw],
                        in0=pu, in1=sg[:], op=OP.mult,
                    )
                else:
                    nc.vector.tensor_tensor(
                        out=hT[:, ct - HT, sc0 : sc0 + scw],
                        in0=pu,
                        in1=gsil[:, ct - HT, sc0 : sc0 + scw],
                        op=OP.mult,
                    )

        def ffn_down_block(g, hT, downw, arouts):
            delta_g = act2.tile([P, GS, c.E], BF16, tag="dg", name="dg", bufs=2)
            for si in range(GS):
                st = g * GS + si
                for he in range(c.E // EW):
                    pd = psum.tile([P, 512], F32, tag="mm", name="mm")[:, :EW]
                    for kt in range(HT):
                        nc.tensor.matmul(
                            pd,
                            lhsT=hT[:, kt, st * P : (st + 1) * P],
                            rhs=downw[:, kt, he * EW : (he + 1) * EW],
                            start=(kt == 0),
                            stop=(kt == HT - 1),
                        )
                    nc.scalar.copy(
                        out=delta_g[:, si, he * EW : (he + 1) * EW], in_=pd
                    )
            arouts.append(ar_issue_one(delta_g))

        # ---------------- main schedule ----------------
        pending = None     # arouts of the previous phase (None for layer 0)
        diag_next = None   # diag for the next group's norm transposes

        for l in range(c.L):
            qkvw = wpool.tile([P, ET, c.QKVC], BF16, tag="qkvw", name="qkvw", bufs=1)
            nc.sync.dma_start(
                qkvw[:], qkvw_d[l].rearrange("(ko p) n -> p ko n", p=P)
            )
            outw = wpool.tile([P, c.E], BF16, tag="outw", name="outw", bufs=1)
            nc.sync.dma_start(outw[:], outw_d[l])
            downw = wpool.tile([P, HT, c.E], BF16, tag="downw", name="downw")
            nc.sync.dma_start(
                downw[:], downw_d[l].rearrange("(ko p) n -> p ko n", p=P)
            )
            upw = wpool.tile([P, ET, 2 * c.HIDC], BF16, tag="upw", name="upw", bufs=1)
            nc.sync.dma_start(
                upw[:], upw_d[l].rearrange("(ko p) n -> p ko n", p=P)
            )

            # ======== attention phase ========
            normT = act2.tile([P, ET, c.S], BF16, tag="big", name="normT")
            vaug = act.tile([P, ST, HC * (D + 1)], BF16, tag="vaug", name="vaug")
            nc.gpsimd.memset(vaug[:], 1.0)
            rqT = act.tile([P, HC, c.S], BF16, tag="rqT", name="rqT")
            rkT = act.tile([P, HC, c.S], BF16, tag="rkT", name="rkT")
            nc.gpsimd.memset(rqT[D:P, :, :], 0.0)
            nc.gpsimd.memset(rkT[D:P, :, :], 0.0)
            atto = act.tile([P, ST, HC * D], BF16, tag="atto", name="atto")
            attoT = act.tile([P, c.S], BF16, tag="attoT", name="attoT")
            arouts_attn = []

            if l == 0:
                diag_next = norm_prologue(0, None)

            for g in range(GN):
                diag_cur = diag_next
                if g + 1 < GN:
                    diag_next = norm_prologue(
                        g + 1, pending[g + 1] if pending is not None else None
                    )
                norm_transposes(g, normT, diag_cur)
                rq, rk = qkv_rope_block(g, normT, qkvw, vaug)
                if g > 0:
                    attention_block(
                        g - 1, rqT, rkT, vaug, atto, attoT, outw, arouts_attn
                    )
                qkT_block(g, rq, rk, rqT, rkT)
            # attention tail: FFN group-0 prologue first (its serial chain
            # overlaps attention(GN-1)'s tensor work), then the last q-group.
            diag_next = norm_prologue(0, arouts_attn[0])
            attention_block(
                GN - 1, rqT, rkT, vaug, atto, attoT, outw, arouts_attn
            )

            # ======== FFN phase ========
            normT2 = act2.tile([P, ET, c.S], BF16, tag="big", name="normT2")
            gsil = act.tile([P, HT, c.S], BF16, tag="pt1", name="gsil")
            hT = act.tile([P, HT, c.S], BF16, tag="pt0", name="hT")
            arouts_ffn = []
            for g in range(GN):
                diag_cur = diag_next
                if g + 1 < GN:
                    diag_next = norm_prologue(g + 1, arouts_attn[g + 1])
                norm_transposes(g, normT2, diag_cur)
                ffn_up_block(g, normT2, upw, gsil, hT)
                if g > 0:
                    ffn_down_block(g - 1, hT, downw, arouts_ffn)
            # FFN tail: next phase's group-0 prologue, then last down block.
            diag_next = norm_prologue(0, arouts_ffn[0])
            ffn_down_block(GN - 1, hT, downw, arouts_ffn)
            pending = arouts_ffn

        # ======== final norm + logits ========
        normTf = act2.tile([P, ET, c.S], BF16, tag="big", name="normTf")
        for g in range(GN):
            diag_cur = diag_next
            if g + 1 < GN:
                diag_next = norm_prologue(g + 1, pending[g + 1])
            norm_transposes(g, normTf, diag_cur)

        copy_engines = [nc.vector, nc.scalar]
        for nn in range(c.VC // c.VCHUNK):
            vw = wpool.tile([P, ET, c.VCHUNK], BF16, tag="vocw", name="vw", bufs=2)
            nc.sync.dma_start(
                vw[:],
                vocw_d[:, nn * c.VCHUNK : (nn + 1) * c.VCHUNK].rearrange(
                    "(ko p) n -> p ko n", p=P
                ),
            )
            for st in range(ST):
                pl = psum.tile([P, 512], F32, tag="mm", name="mm")[:, : c.VCHUNK]
                for kk in range(ET):
                    nc.tensor.matmul(
                        pl,
                        lhsT=normTf[:, kk, st * P : (st + 1) * P],
                        rhs=vw[:, kk, :],
                        start=(kk == 0),
                        stop=(kk == ET - 1),
                    )
                lo = work.tile([P, c.VCHUNK], BF16, tag="lo", name="lo", bufs=4)
                if st % 2 == 0:
                    nc.scalar.copy(out=lo[:], in_=pl)
                else:
                    nc.vector.tensor_copy(out=lo[:], in_=pl)
                nc.sync.dma_start(
                    out_d[st * P : (st + 1) * P,
                          nn * c.VCHUNK : (nn + 1) * c.VCHUNK],
                    lo[:],
                )

    nc.compile()
    return nc


# ---------------- host side ----------------

def _rope_consts(cfg: Cfg):
    S, D = cfg.S, cfg.D
    half = D // 2
    i = np.arange(D)
    offset = i % half
    scales = np.power(10000.0, (-2.0 / D) * offset.astype(np.float32))
    m = np.arange(S, dtype=np.float32)
    angles = m[:, None] * scales[None, :]
    cos = np.cos(angles).astype(np.float32)
    sin = np.sin(angles).astype(np.float32)
    sin_eff = np.concatenate([-sin[:, :half], sin[:, half:]], axis=-1)

    def to_tile(a):  # [S, D] -> [P, ST*D]
        return (
            a.reshape(cfg.ST, P, D).transpose(1, 0, 2).reshape(P, cfg.ST * D)
        )

    bf = ml_dtypes.bfloat16
    return to_tile(cos).astype(bf), to_tile(sin_eff).astype(bf)


def make_in_maps(cfg: Cfg, tokens, table, qkv_w, out_w, up_w, down_w, vocab_w):
    c = cfg
    bf = ml_dtypes.bfloat16
    HD = c.HC * c.D        # head-dim cols per core
    H_ALL = c.n_cores * c.HC
    HID_ALL = c.n_cores * c.HIDC

    tokens = np.asarray(tokens).reshape(-1)
    idx = tokens.reshape(c.ST, P).T.astype(np.int32).copy()  # [P, ST]

    table = np.asarray(table, dtype=np.float32).copy()
    table[0] = 0.0
    table_bf = table.astype(bf)

    cos_t, sin_t = _rope_consts(c)
    qscale = 1.0 / math.sqrt(c.D)

    qkv_w = np.asarray(qkv_w, dtype=np.float32)
    out_w = np.asarray(out_w, dtype=np.float32)
    up_w = np.asarray(up_w, dtype=np.float32)
    down_w = np.asarray(down_w, dtype=np.float32)
    vocab_w = np.asarray(vocab_w, dtype=np.float32)

    in_maps = []
    for core in range(c.n_cores):
        hlo = core * HD
        q_cols = slice(hlo, hlo + HD)
        k_cols = slice(H_ALL * c.D + hlo, H_ALL * c.D + hlo + HD)
        v_cols = slice(2 * H_ALL * c.D + hlo, 2 * H_ALL * c.D + hlo + HD)
        qkv_c = np.concatenate(
            [qkv_w[:, :, q_cols] * qscale, qkv_w[:, :, k_cols], qkv_w[:, :, v_cols]],
            axis=2,
        ).astype(bf)
        out_c = out_w[:, hlo : hlo + HD, :].astype(bf)
        g_cols = slice(core * c.HIDC, (core + 1) * c.HIDC)
        u_cols = slice(HID_ALL + core * c.HIDC, HID_ALL + (core + 1) * c.HIDC)
        up_c = np.concatenate([up_w[:, :, g_cols], up_w[:, :, u_cols]], axis=2).astype(bf)
        down_c = down_w[:, core * c.HIDC : (core + 1) * c.HIDC, :].astype(bf)
        voc_c = vocab_w[:, core * c.VC : (core + 1) * c.VC].astype(bf)
        in_maps.append(
            {
                "idx": idx,
                "table": table_bf,
                "qkvw": np.ascontiguousarray(qkv_c),
                "outw": np.ascontiguousarray(out_c),
                "upw": np.ascontiguousarray(up_c),
                "downw": np.ascontiguousarray(down_c),
                "vocw": np.ascontiguousarray(voc_c),
                "cos": cos_t,
                "sin": sin_t,
            }
        )
    return in_maps


LAST_EXEC_TIME_NS = None
LAST_RESULTS = None


def kernel(tokens, table, qkv_w, out_w, up_w, down_w, vocab_w):
    global LAST_EXEC_TIME_NS, LAST_RESULTS
    cfg = Cfg()
    if os.environ.get("BASS_TRACE"):
        try:  # antenv.axon_hooks is missing in this image; provide it
            import types
            import antenv

            if "antenv.axon_hooks" not in sys.modules:
                mod = types.ModuleType("antenv.axon_hooks")
                mod._hook = None
                mod.set_axon_ntff_profile_hook = lambda h: setattr(mod, "_hook", h)
                mod.get_axon_ntff_profile_hook = lambda: mod._hook
                sys.modules["antenv.axon_hooks"] = mod
                antenv.axon_hooks = mod
                from trn_agent_boot.trn_boot import _ntff_profile_via_ctypes

                mod.set_axon_ntff_profile_hook(
                    _ntff_profile_via_ctypes("/opt/axon/libaxon_pjrt.so")
                )
        except Exception as e:
            print(f"[kernel] trace hook setup failed: {e}", file=sys.stderr)

    nc = build_kernel(cfg)
    in_maps = make_in_maps(cfg, tokens, table, qkv_w, out_w, up_w, down_w, vocab_w)
    res = run_bass_kernel_spmd(
        nc, in_maps, core_ids=list(range(cfg.n_cores)),
        trace=bool(os.environ.get("BASS_TRACE")),
    )
    LAST_EXEC_TIME_NS = res.exec_time_ns
    LAST_RESULTS = res
    logits = np.concatenate([r["out"] for r in res.results], axis=1)
    return logits[None].astype(np.float32)


# revision 10
# speedup vs baseline: 1.1688x; 1.1688x over previous
"""Trainium2 Bass kernel: 4-layer dense transformer (Megatron TP over 8 NeuronCores).

Sharding (per sharding hint):
  - qkv_w / up_w sharded along output dim, out_w / down_w along input dim.
  - Heads: 16 heads / 8 cores = 2 heads per core.
  - vocab_w sharded along vocab dim (4000 cols/core); host concatenates logits.
  - Norms + residual replicated; deltas AllReduced (bf16) after out-proj / down-proj.

Device schedule: fully software-pipelined. The sequence dim is split into GN=4
groups (the causal attention q-chunks). Each phase (attn / ffn / final norm) is
a stream of per-group macro-steps; the serial prologue chain for group g+1
(AllReduce collect -> residual add -> sum-of-squares -> rsqrt -> diag(scale))
is issued one macro-step EARLY, so it runs on the vector/scalar/gpsimd engines
while the tensor engine is busy with group g's matmuls. Attention consumption
lags production by one group (attention(g-1) is issued between qkv(g) and the
q/k transposes of g) so the rope vector work never stalls the PE. This keeps
the PE continuously fed, which also keeps it at max p-state (2.4 GHz vs 1.2).
"""

import os
import sys

if "/opt/trn_rl_repo" not in sys.path:
    sys.path.insert(0, "/opt/trn_rl_repo")

import math
from dataclasses import dataclass

import numpy as np
import ml_dtypes

import concourse.bass as bass
import concourse.bacc as bacc
import concourse.mybir as mybir
import concourse.tile as tile
from concourse.bass import IndirectOffsetOnAxis
from concourse.bass_utils import run_bass_kernel_spmd
from concourse.masks import make_identity

F32 = mybir.dt.float32
BF16 = mybir.dt.bfloat16
I32 = mybir.dt.int32
AF = mybir.ActivationFunctionType
OP = mybir.AluOpType

P = 128


@dataclass(frozen=True)
class Cfg:
    S: int = 2048        # sequence
    E: int = 1024        # embed
    D: int = 64          # head dim
    HC: int = 2          # heads per core
    HIDC: int = 512      # hidden shard per core (of 4096 / 8)
    VC: int = 4000       # vocab shard per core
    L: int = 4           # layers
    n_cores: int = 8
    V_ROWS: int = 32000  # embedding table rows
    eps: float = 1e-5

    @property
    def ST(self):
        return self.S // P

    @property
    def ET(self):
        return self.E // P

    @property
    def HT(self):
        return self.HIDC // P

    @property
    def QKVC(self):
        return 3 * self.HC * self.D

    @property
    def QC(self):
        return min(512, self.S)  # q chunk for attention

    @property
    def VCHUNK(self):
        return 250


def build_kernel(cfg: Cfg):
    c = cfg
    ST, ET, HT = c.ST, c.ET, c.HT
    D, HC = c.D, c.HC
    D2 = D // 2
    EG = min(4, ET)
    GN = max(c.S // c.QC, 1)   # 4 groups
    GS = ST // GN              # 4 s-tiles per group
    EW = min(512, c.E)

    nc = bacc.Bacc(
        "TRN2", target_bir_lowering=False, debug=False, num_devices=cfg.n_cores
    )

    idx_d = nc.declare_dram_parameter("idx", [P, ST], I32, isOutput=False)
    table_d = nc.declare_dram_parameter("table", [c.V_ROWS, c.E], BF16, isOutput=False)
    qkvw_d = nc.declare_dram_parameter("qkvw", [c.L, c.E, c.QKVC], BF16, isOutput=False)
    outw_d = nc.declare_dram_parameter("outw", [c.L, HC * D, c.E], BF16, isOutput=False)
    upw_d = nc.declare_dram_parameter("upw", [c.L, c.E, 2 * c.HIDC], BF16, isOutput=False)
    downw_d = nc.declare_dram_parameter("downw", [c.L, c.HIDC, c.E], BF16, isOutput=False)
    vocw_d = nc.declare_dram_parameter("vocw", [c.E, c.VC], BF16, isOutput=False)
    cos_d = nc.declare_dram_parameter("cos", [P, ST * D], BF16, isOutput=False)
    sin_d = nc.declare_dram_parameter("sin", [P, ST * D], BF16, isOutput=False)
    out_d = nc.declare_dram_parameter("out", [c.S, c.VC], BF16, isOutput=True)

    from contextlib import ExitStack

    with tile.TileContext(nc) as tc, ExitStack() as es:
        const = es.enter_context(tc.tile_pool(name="const", bufs=1))
        wpool = es.enter_context(tc.tile_pool(name="w", bufs=1))
        act = es.enter_context(tc.tile_pool(name="act", bufs=1))
        act2 = es.enter_context(tc.tile_pool(name="act2", bufs=1))
        work = es.enter_context(tc.tile_pool(name="work", bufs=2))
        psum = es.enter_context(tc.tile_pool(name="psum", bufs=5, space="PSUM"))
        psum_tr = es.enter_context(tc.tile_pool(name="psumtr", bufs=3, space="PSUM"))
        dram = es.enter_context(tc.tile_pool(name="dram", bufs=8, space="DRAM"))

        ident = const.tile([P, P], BF16, tag="ident")
        make_identity(nc, ident[:])
        rope_tiles = {}
        for name, d in (("cos", cos_d), ("sin", sin_d)):
            t = const.tile([P, ST, D], BF16, tag=name, name=name)
            nc.sync.dma_start(t[:], d[:].rearrange("p (st d) -> p st d", d=D))
            rope_tiles[name] = t
        cosw, sinw = rope_tiles["cos"], rope_tiles["sin"]
        idx_sb = const.tile([P, ST], I32, tag="idx")
        nc.sync.dma_start(idx_sb[:], idx_d[:])

        emb = act.tile([P, ST, c.E], BF16, tag="emb")
        for st in range(ST):
            nc.gpsimd.indirect_dma_start(
                out=emb[:, st, :],
                out_offset=None,
                in_=table_d[:],
                in_offset=IndirectOffsetOnAxis(ap=idx_sb[:, st : st + 1], axis=0),
            )

        # ---------------- helpers ----------------

        def norm_prologue(g, arout_src):
            """Collect AR delta for group g into emb, compute the rmsnorm
            scale, and build diag(scale) blocks. Issued one macro-step before
            the tensor-engine consumer (norm_transposes)."""
            gsl = slice(g * GS, (g + 1) * GS)
            if arout_src is not None:
                gath = act2.tile([P, GS, c.E], BF16, tag="dg", name="gath", bufs=2)
                nc.sync.dma_start(
                    gath[:],
                    arout_src[:].rearrange("(st p) e -> p st e", p=P),
                )
                nc.vector.tensor_tensor(
                    out=emb[:, gsl, :], in0=emb[:, gsl, :], in1=gath[:], op=OP.add
                )
            ssq_g = work.tile([P, GS], F32, tag="ssqg", name="ssq_g")
            for si in range(GS):
                st = g * GS + si
                scr = work.tile([P, c.E], BF16, tag="sqscr", name="scr", bufs=1)
                nc.scalar.activation(
                    out=scr[:], in_=emb[:, st, :], func=AF.Square,
                    accum_out=ssq_g[:, si : si + 1],
                )
            ms = work.tile([P, GS], F32, tag="ms", name="ms")
            nc.vector.tensor_scalar(
                ms[:], ssq_g[:], 1.0 / c.E, c.eps, OP.mult, OP.add
            )
            rms = work.tile([P, GS], F32, tag="rms", name="rms")
            nc.scalar.activation(out=rms[:], in_=ms[:], func=AF.Sqrt)
            scale = work.tile([P, GS], F32, tag="scale", name="scale")
            nc.vector.reciprocal(out=scale[:], in_=rms[:])
            scale_b = work.tile([P, GS], BF16, tag="scalebg", name="scale_b")
            nc.vector.tensor_copy(out=scale_b[:], in_=scale[:])
            diag = work.tile([P, GS, P], BF16, tag="diag", name="diag", bufs=2)
            for si in range(GS):
                nc.gpsimd.affine_select(
                    out=diag[:, si, :],
                    in_=scale_b[:, si : si + 1].to_broadcast((P, P)),
                    pattern=[[-1, P]],
                    compare_op=OP.is_equal,
                    fill=0.0,
                    base=0,
                    channel_multiplier=1,
                )
            return diag

        def norm_transposes(g, normT, diag):
            """PE-transpose emb s-tiles of group g with the rmsnorm scale
            folded in via diag(scale)."""
            for si in range(GS):
                st = g * GS + si
                for eg in range(ET // EG):
                    ptr = psum_tr.tile([P, 512], F32, tag="tr", name="tr")[:, : EG * P]
                    for j in range(EG):
                        ee = eg * EG + j
                        nc.tensor.matmul(
                            ptr[:, j * P : (j + 1) * P],
                            lhsT=emb[:, st, ee * P : (ee + 1) * P],
                            rhs=diag[:, si, :],
                            start=True,
                            stop=True,
                        )
                    nc.vector.tensor_copy(
                        out=normT[:, eg * EG : (eg + 1) * EG,
                                  st * P : (st + 1) * P],
                        in_=ptr.rearrange("p (g q) -> p g q", g=EG),
                    )

        def qkv_rope_block(g, normT, qkvw, vaug):
            qkvn = act.tile([P, GS, c.QKVC], BF16, tag="qg", name="qkvn", bufs=2)
            rq = act.tile([P, GS, HC * D], BF16, tag="ra", name="rq", bufs=2)
            rk = act.tile([P, GS, HC * D], BF16, tag="rb", name="rk", bufs=2)
            HGS = GS // 2
            for half in range(2):
                for si in range(half * HGS, (half + 1) * HGS):
                    st = g * GS + si
                    pq = psum.tile([P, 512], F32, tag="mm", name="mm")[:, : c.QKVC]
                    for kk in range(ET):
                        nc.tensor.matmul(
                            pq,
                            lhsT=normT[:, kk, st * P : (st + 1) * P],
                            rhs=qkvw[:, kk, :],
                            start=(kk == 0),
                            stop=(kk == ET - 1),
                        )
                    nc.scalar.copy(out=qkvn[:, si, :], in_=pq)
                hl = slice(half * HGS, (half + 1) * HGS)
                hg = slice(g * GS + half * HGS, g * GS + (half + 1) * HGS)
                for (src0, dst) in ((0, rq), (HC * D, rk)):
                    for h in range(HC):
                        s0 = src0 + h * D
                        tmp = work.tile([P, HGS, D], BF16, tag="ropetmp",
                                        name="tmp", bufs=2)
                        nc.vector.tensor_tensor(
                            out=tmp[:, :, 0:D2],
                            in0=qkvn[:, hl, s0 + D2 : s0 + D],
                            in1=sinw[:, hg, 0:D2],
                            op=OP.mult,
                        )
                        nc.vector.tensor_tensor(
                            out=tmp[:, :, D2:D],
                            in0=qkvn[:, hl, s0 : s0 + D2],
                            in1=sinw[:, hg, D2:D],
                            op=OP.mult,
                        )
                        o = dst[:, hl, h * D : (h + 1) * D]
                        nc.vector.tensor_tensor(
                            out=o, in0=qkvn[:, hl, s0 : s0 + D],
                            in1=cosw[:, hg], op=OP.mult)
                        nc.vector.tensor_tensor(out=o, in0=o, in1=tmp[:], op=OP.add)
                for h in range(HC):
                    nc.vector.tensor_copy(
                        out=vaug[:, hg, h * (D + 1) : h * (D + 1) + D],
                        in_=qkvn[:, hl,
                                 2 * HC * D + h * D : 2 * HC * D + (h + 1) * D],
                    )
            return rq, rk

        def qkT_block(g, rq, rk, rqT, rkT):
            for si in range(GS):
                st = g * GS + si
                ptr = psum_tr.tile([P, 512], F32, tag="tr", name="tr")
                for h in range(HC):
                    nc.tensor.matmul(
                        ptr[0:D, h * P : (h + 1) * P],
                        lhsT=rq[:, si, h * D : (h + 1) * D],
                        rhs=ident[:], start=True, stop=True)
                    nc.tensor.matmul(
                        ptr[0:D, (HC + h) * P : (HC + h + 1) * P],
                        lhsT=rk[:, si, h * D : (h + 1) * D],
                        rhs=ident[:], start=True, stop=True)
                for h in range(HC):
                    nc.vector.tensor_copy(
                        out=rqT[0:D, h, st * P : (st + 1) * P],
                        in_=ptr[0:D, h * P : (h + 1) * P])
                    nc.vector.tensor_copy(
                        out=rkT[0:D, h, st * P : (st + 1) * P],
                        in_=ptr[0:D, (HC + h) * P : (HC + h + 1) * P])

        def ar_issue_one(delta_g, arin):
            arout = dram.tile(
                [GS * P, c.E], BF16, tag="arout", name="arout",
                addr_space="Shared" if c.n_cores > 4 else "Local",
            )
            nc.gpsimd.collective_compute(
                "AllReduce",
                OP.add,
                replica_groups=[list(range(c.n_cores))],
                ins=[arin[:].opt()],
                outs=[arout[:].opt()],
            )
            return arout

        def attention_block(g, rqT, rkT, vaug, atto, attoT, outw, arouts):
            """scores + softmax + AV + out-proj + AR issue for q-group g.
            Both heads' score matmuls are issued before either head's AV so
            the Exp/mask chain of head h overlaps the PE work of head 1-h."""
            q0 = g * c.QC
            nk = (q0 + c.QC) // P
            PTs = []
            for h in range(HC):
                PT = act.tile([P, ST, c.QC], BF16, tag=f"pt{h}", name=f"pt{h}")
                PTs.append(PT)
                for kt in range(nk):
                    ps = psum.tile([P, 512], F32, tag="mm", name="mm")[:, : c.QC]
                    nc.tensor.matmul(
                        ps,
                        lhsT=rkT[:, h, kt * P : (kt + 1) * P],
                        rhs=rqT[:, h, q0 : q0 + c.QC],
                        start=True,
                        stop=True,
                    )
                    nc.scalar.activation(out=PT[:, kt, :], in_=ps, func=AF.Exp)
                    if (kt + 1) * P > q0:
                        nc.gpsimd.affine_select(
                            out=PT[:, kt, :],
                            in_=PT[:, kt, :],
                            pattern=[[1, c.QC]],
                            compare_op=OP.is_ge,
                            fill=0.0,
                            base=q0 - kt * P,
                            channel_multiplier=-1,
                        )
            for h in range(HC):
                PT = PTs[h]
                for qs in range(c.QC // P):
                    po = psum.tile([P, 512], F32, tag="mm", name="mm")[:, : D + 1]
                    for kt in range(nk):
                        nc.tensor.matmul(
                            po,
                            lhsT=PT[:, kt, qs * P : (qs + 1) * P],
                            rhs=vaug[:, kt, h * (D + 1) : (h + 1) * (D + 1)],
                            start=(kt == 0),
                            stop=(kt == nk - 1),
                        )
                    st_out = (q0 // P) + qs
                    rec = work.tile([P, 1], F32, tag="rec", name="rec", bufs=1)
                    nc.vector.reciprocal(out=rec[:], in_=po[:, D : D + 1])
                    nc.vector.tensor_scalar_mul(
                        atto[:, st_out, h * D : (h + 1) * D], po[:, 0:D], rec[:]
                    )
            # attoT + out-proj + AR issue
            delta_g = act2.tile([P, GS, c.E], BF16, tag="dg", name="dg", bufs=2)
            arin = dram.tile([GS * P, c.E], BF16, tag="arin", name="arin")
            for si in range(GS):
                st = g * GS + si
                ptr = psum_tr.tile([P, 512], F32, tag="tr", name="tr")[:, :P]
                nc.tensor.matmul(ptr, lhsT=atto[:, st, :], rhs=ident[:],
                                 start=True, stop=True)
                nc.vector.tensor_copy(
                    out=attoT[:, st * P : (st + 1) * P], in_=ptr
                )
                for he in range(c.E // EW):
                    pd = psum.tile([P, 512], F32, tag="mm", name="mm")[:, :EW]
                    nc.tensor.matmul(
                        pd,
                        lhsT=attoT[:, st * P : (st + 1) * P],
                        rhs=outw[:, he * EW : (he + 1) * EW],
                        start=True,
                        stop=True,
                    )
                    nc.scalar.copy(
                        out=delta_g[:, si, he * EW : (he + 1) * EW], in_=pd
                    )
                nc.sync.dma_start(
                    arin[si * P : (si + 1) * P, :], delta_g[:, si, :]
                )
            arouts.append(ar_issue_one(delta_g, arin))

        def ffn_up_block(g, normT2, upw, gsil, hT):
            sc0 = g * GS * P
            scw = GS * P
            for ct in range(2 * HT):
                pu = psum.tile([P, 512], F32, tag="mm", name="mm")[:, :scw]
                for kk in range(ET):
                    nc.tensor.matmul(
                        pu,
                        lhsT=upw[:, kk, ct * P : (ct + 1) * P],
                        rhs=normT2[:, kk, sc0 : sc0 + scw],
                        start=(kk == 0),
                        stop=(kk == ET - 1),
                    )
                if ct < HT:
                    sg = work.tile([P, scw], BF16, tag="sg", name="sg", bufs=1)
                    nc.scalar.activation(out=sg[:], in_=pu, func=AF.Sigmoid)
                    nc.vector.tensor_tensor(
                        out=gsil[:, ct, sc0 : sc0 + scw],
                        in0=pu, in1=sg[:], op=OP.mult,
                    )
                else:
                    nc.vector.tensor_tensor(
                        out=hT[:, ct - HT, sc0 : sc0 + scw],
                        in0=pu,
                        in1=gsil[:, ct - HT, sc0 : sc0 + scw],
                        op=OP.mult,
                    )

        def ffn_down_block(g, hT, downw, arouts):
            delta_g = act2.tile([P, GS, c.E], BF16, tag="dg", name="dg", bufs=2)
            arin = dram.tile([GS * P, c.E], BF16, tag="arin", name="arin")
            for si in range(GS):
                st = g * GS + si
                for he in range(c.E // EW):
                    pd = psum.tile([P, 512], F32, tag="mm", name="mm")[:, :EW]
                    for kt in range(HT):
                        nc.tensor.matmul(
                            pd,
                            lhsT=hT[:, kt, st * P : (st + 1) * P],
                            rhs=downw[:, kt, he * EW : (he + 1) * EW],
                            start=(kt == 0),
                            stop=(kt == HT - 1),
                        )
                    nc.scalar.copy(
                        out=delta_g[:, si, he * EW : (he + 1) * EW], in_=pd
                    )
                nc.sync.dma_start(
                    arin[si * P : (si + 1) * P, :], delta_g[:, si, :]
                )
            arouts.append(ar_issue_one(delta_g, arin))

        # ---------------- main schedule ----------------
        pending = None     # arouts of the previous phase (None for layer 0)
        diag_next = None   # diag for the next group's norm transposes

        for l in range(c.L):
            qkvw = wpool.tile([P, ET, c.QKVC], BF16, tag="qkvw", name="qkvw", bufs=1)
            nc.sync.dma_start(
                qkvw[:], qkvw_d[l].rearrange("(ko p) n -> p ko n", p=P)
            )
            outw = wpool.tile([P, c.E], BF16, tag="outw", name="outw", bufs=1)
            nc.sync.dma_start(outw[:], outw_d[l])
            downw = wpool.tile([P, HT, c.E], BF16, tag="downw", name="downw")
            nc.sync.dma_start(
                downw[:], downw_d[l].rearrange("(ko p) n -> p ko n", p=P)
            )
            upw = wpool.tile([P, ET, 2 * c.HIDC], BF16, tag="upw", name="upw", bufs=1)
            nc.sync.dma_start(
                upw[:], upw_d[l].rearrange("(ko p) n -> p ko n", p=P)
            )

            # ======== attention phase ========
            normT = act2.tile([P, ET, c.S], BF16, tag="big", name="normT")
            vaug = act.tile([P, ST, HC * (D + 1)], BF16, tag="vaug", name="vaug")
            nc.gpsimd.memset(vaug[:], 1.0)
            rqT = act.tile([P, HC, c.S], BF16, tag="rqT", name="rqT")
            rkT = act.tile([P, HC, c.S], BF16, tag="rkT", name="rkT")
            nc.gpsimd.memset(rqT[D:P, :, :], 0.0)
            nc.gpsimd.memset(rkT[D:P, :, :], 0.0)
            atto = act.tile([P, ST, HC * D], BF16, tag="atto", name="atto")
            attoT = act.tile([P, c.S], BF16, tag="attoT", name="attoT")
            arouts_attn = []

            if l == 0:
                diag_next = norm_prologue(0, None)

            for g in range(GN):
                diag_cur = diag_next
                if g + 1 < GN:
                    diag_next = norm_prologue(
                        g + 1, pending[g + 1] if pending is not None else None
                    )
                norm_transposes(g, normT, diag_cur)
                rq, rk = qkv_rope_block(g, normT, qkvw, vaug)
                if g > 0:
                    attention_block(
                        g - 1, rqT, rkT, vaug, atto, attoT, outw, arouts_attn
                    )
                qkT_block(g, rq, rk, rqT, rkT)
            # attention tail: FFN group-0 prologue first (its serial chain
            # overlaps attention(GN-1)'s tensor work), then the last q-group.
            diag_next = norm_prologue(0, arouts_attn[0])
            attention_block(
                GN - 1, rqT, rkT, vaug, atto, attoT, outw, arouts_attn
            )

            # ======== FFN phase ========
            normT2 = act2.tile([P, ET, c.S], BF16, tag="big", name="normT2")
            gsil = act.tile([P, HT, c.S], BF16, tag="pt1", name="gsil")
            hT = act.tile([P, HT, c.S], BF16, tag="pt0", name="hT")
            arouts_ffn = []
            for g in range(GN):
                diag_cur = diag_next
                if g + 1 < GN:
                    diag_next = norm_prologue(g + 1, arouts_attn[g + 1])
                norm_transposes(g, normT2, diag_cur)
                ffn_up_block(g, normT2, upw, gsil, hT)
                if g > 0:
                    ffn_down_block(g - 1, hT, downw, arouts_ffn)
            # FFN tail: next phase's group-0 prologue, then last down block.
            diag_next = norm_prologue(0, arouts_ffn[0])
            ffn_down_block(GN - 1, hT, downw, arouts_ffn)
            pending = arouts_ffn

        # ======== final norm + logits ========
        normTf = act2.tile([P, ET, c.S], BF16, tag="big", name="normTf")
        for g in range(GN):
            diag_cur = diag_next
            if g + 1 < GN:
                diag_next = norm_prologue(g + 1, pending[g + 1])
            norm_transposes(g, normTf, diag_cur)

        copy_engines = [nc.vector, nc.scalar]
        for nn in range(c.VC // c.VCHUNK):
            vw = wpool.tile([P, ET, c.VCHUNK], BF16, tag="vocw", name="vw", bufs=2)
            nc.sync.dma_start(
                vw[:],
                vocw_d[:, nn * c.VCHUNK : (nn + 1) * c.VCHUNK].rearrange(
                    "(ko p) n -> p ko n", p=P
                ),
            )
            for st in range(ST):
                pl = psum.tile([P, 512], F32, tag="mm", name="mm")[:, : c.VCHUNK]
                for kk in range(ET):
                    nc.tensor.matmul(
                        pl,
                        lhsT=normTf[:, kk, st * P : (st + 1) * P],
                        rhs=vw[:, kk, :],
                        start=(kk == 0),
                        stop=(kk == ET - 1),
                    )
                lo = work.tile([P, c.VCHUNK], BF16, tag="lo", name="lo", bufs=4)
                if st % 2 == 0:
                    nc.scalar.copy(out=lo[:], in_=pl)
                else:
                    nc.vector.tensor_copy(out=lo[:], in_=pl)
                nc.sync.dma_start(
                    out_d[st * P : (st + 1) * P,
                          nn * c.VCHUNK : (nn + 1) * c.VCHUNK],
                    lo[:],
                )

    nc.compile()
    return nc


# ---------------- host side ----------------

def _rope_consts(cfg: Cfg):
    S, D = cfg.S, cfg.D
    half = D // 2
    i = np.arange(D)
    offset = i % half
    scales = np.power(10000.0, (-2.0 / D) * offset.astype(np.float32))
    m = np.arange(S, dtype=np.float32)
    angles = m[:, None] * scales[None, :]
    cos = np.cos(angles).astype(np.float32)
    sin = np.sin(angles).astype(np.float32)
    sin_eff = np.concatenate([-sin[:, :half], sin[:, half:]], axis=-1)

    def to_tile(a):  # [S, D] -> [P, ST*D]
        return (
            a.reshape(cfg.ST, P, D).transpose(1, 0, 2).reshape(P, cfg.ST * D)
        )

    bf = ml_dtypes.bfloat16
    return to_tile(cos).astype(bf), to_tile(sin_eff).astype(bf)


def make_in_maps(cfg: Cfg, tokens, table, qkv_w, out_w, up_w, down_w, vocab_w):
    c = cfg
    bf = ml_dtypes.bfloat16
    HD = c.HC * c.D        # head-dim cols per core
    H_ALL = c.n_cores * c.HC
    HID_ALL = c.n_cores * c.HIDC

    tokens = np.asarray(tokens).reshape(-1)
    idx = tokens.reshape(c.ST, P).T.astype(np.int32).copy()  # [P, ST]

    table = np.asarray(table, dtype=np.float32).copy()
    table[0] = 0.0
    table_bf = table.astype(bf)

    cos_t, sin_t = _rope_consts(c)
    qscale = 1.0 / math.sqrt(c.D)

    qkv_w = np.asarray(qkv_w, dtype=np.float32)
    out_w = np.asarray(out_w, dtype=np.float32)
    up_w = np.asarray(up_w, dtype=np.float32)
    down_w = np.asarray(down_w, dtype=np.float32)
    vocab_w = np.asarray(vocab_w, dtype=np.float32)

    in_maps = []
    for core in range(c.n_cores):
        hlo = core * HD
        q_cols = slice(hlo, hlo + HD)
        k_cols = slice(H_ALL * c.D + hlo, H_ALL * c.D + hlo + HD)
        v_cols = slice(2 * H_ALL * c.D + hlo, 2 * H_ALL * c.D + hlo + HD)
        qkv_c = np.concatenate(
            [qkv_w[:, :, q_cols] * qscale, qkv_w[:, :, k_cols], qkv_w[:, :, v_cols]],
            axis=2,
        ).astype(bf)
        out_c = out_w[:, hlo : hlo + HD, :].astype(bf)
        g_cols = slice(core * c.HIDC, (core + 1) * c.HIDC)
        u_cols = slice(HID_ALL + core * c.HIDC, HID_ALL + (core + 1) * c.HIDC)
        up_c = np.concatenate([up_w[:, :, g_cols], up_w[:, :, u_cols]], axis=2).astype(bf)
        down_c = down_w[:, core * c.HIDC : (core + 1) * c.HIDC, :].astype(bf)
        voc_c = vocab_w[:, core * c.VC : (core + 1) * c.VC].astype(bf)
        in_maps.append(
            {
                "idx": idx,
                "table": table_bf,
                "qkvw": np.ascontiguousarray(qkv_c),
                "outw": np.ascontiguousarray(out_c),
                "upw": np.ascontiguousarray(up_c),
                "downw": np.ascontiguousarray(down_c),
                "vocw": np.ascontiguousarray(voc_c),
                "cos": cos_t,
                "sin": sin_t,
            }
        )
    return in_maps


LAST_EXEC_TIME_NS = None
LAST_RESULTS = None


def kernel(tokens, table, qkv_w, out_w, up_w, down_w, vocab_w):
    global LAST_EXEC_TIME_NS, LAST_RESULTS
    cfg = Cfg()
    if os.environ.get("BASS_TRACE"):
        try:  # antenv.axon_hooks is missing in this image; provide it
            import types
            import antenv

            if "antenv.axon_hooks" not in sys.modules:
                mod = types.ModuleType("antenv.axon_hooks")
                mod._hook = None
                mod.set_axon_ntff_profile_hook = lambda h: setattr(mod, "_hook", h)
                mod.get_axon_ntff_profile_hook = lambda: mod._hook
                sys.modules["antenv.axon_hooks"] = mod
                antenv.axon_hooks = mod
                from trn_agent_boot.trn_boot import _ntff_profile_via_ctypes

                mod.set_axon_ntff_profile_hook(
                    _ntff_profile_via_ctypes("/opt/axon/libaxon_pjrt.so")
                )
        except Exception as e:
            print(f"[kernel] trace hook setup failed: {e}", file=sys.stderr)

    nc = build_kernel(cfg)
    in_maps = make_in_maps(cfg, tokens, table, qkv_w, out_w, up_w, down_w, vocab_w)
    res = run_bass_kernel_spmd(
        nc, in_maps, core_ids=list(range(cfg.n_cores)),
        trace=bool(os.environ.get("BASS_TRACE")),
    )
    LAST_EXEC_TIME_NS = res.exec_time_ns
    LAST_RESULTS = res
    logits = np.concatenate([r["out"] for r in res.results], axis=1)
    return logits[None].astype(np.float32)
